# revision 64
# baseline (speedup 1.0000x reference)
"""Two-layer GCN (GCNConv x2, PyG-style symmetric normalization) on 8 trn2
NeuronCores.

Strategy v3 (push-side vertex-cut + ReduceScatter, both layers):
  - Edges (incl. host-added self-loops) are partitioned by SOURCE bucket:
    core c owns all edges whose source node lives in its 12.5k-node
    bucket.  Each core transforms only its own x rows (g1 = dis*(x@W1),
    a 3.2 MB local bf16 table), gathers message rows from that LOCAL
    table (single int16-addressable piece, SWDGE dma_gather), and
    scatter-accumulates per-destination partial sums over ALL 800 global
    destination blocks via one-hot-mask matmuls in PSUM supers.
  - Partial sums (bf16) are written to DRAM [102400 x C] and a single
    ReduceScatter(add) hands every core the complete aggregation for its
    own bucket - collective output is only 3.3/1.6 MB, far cheaper than
    an AllGather of the full feature table.
  - Post-RS (layer 1): h1 = relu(dis*rs + b1) per block, then
    g2 = dis*(h1@W2) via PE transpose + matmul, written to the local
    layer-2 table.  Layer 2 aggregates with the SAME gather planes
    (same edges, same tiles, same masks) and finishes with a second
    ReduceScatter and out = dis*rs + b2.
  - Destination supers of 8 blocks share tiles: a 128-edge tile spans
    1-2 adjacent dst blocks (edges sorted by destination), so tile
    padding is ~10% instead of ~25%.  Mask instances are the union over
    cores of blocks each tile touches; per-core rel planes mark foreign
    slots with -1 (mask row all zero).

Host-side work is limited to index plumbing: bucketing/sorting edges,
building gather-index/mask-value planes, degree counts and data layout
(transpose/pad).  All floating-point math (1/sqrt, matmuls, scaling,
bias, relu) runs on device.
"""

import math

import numpy as np

CFG_FULL = dict(N=100000, E=1600000, CIN=128, CHID=128, COUT=64)

NCORES = 8
SUPER = 8    # dst blocks per PSUM super
SBATCH = 4   # supers per gather batch


def _derive(cfg):
    n = cfg["N"]
    bucket = n // NCORES            # 12500 nodes per core
    assert bucket * NCORES == n
    blocks = math.ceil(bucket / 128)
    blocks = math.ceil(blocks / 4) * 4   # 100 own blocks
    shard = blocks * 128            # 12800 padded rows per core
    assert shard <= 32600           # int16 gather index limit
    tot_rows = shard * NCORES       # 102400 global rows
    tot_blocks = tot_rows // 128    # 800 global dst blocks
    n_supers = tot_blocks // SUPER  # 100
    assert n_supers * SUPER == tot_blocks
    return dict(bucket=bucket, blocks=blocks, shard=shard,
                tot_rows=tot_rows, tot_blocks=tot_blocks, n_supers=n_supers)


def _preprocess(edge_index, cfg):
    """Partition edges by source bucket; build shared gather/mask planes."""
    d = _derive(cfg)
    bucket, shard = d["bucket"], d["shard"]
    n_supers = d["n_supers"]
    # self-loops are NOT added as edges: they would all land on the owner
    # core and inflate the shared max-over-cores slot padding by ~50%.
    # The self contribution is added post-ReduceScatter from the local
    # g table instead.
    row = edge_index[0].astype(np.int64)  # src
    col = edge_index[1].astype(np.int64)  # dst

    c_src = row // bucket
    src_loc = row - c_src * bucket           # local table row (< 12500)
    g_dst = (col // bucket) * shard + col % bucket
    g_blk = g_dst // 128
    tot_blocks = d["tot_blocks"]

    # shared slot layout: per dst block, max-over-cores count of slots;
    # all cores use the same block->slot-range mapping, so mask instances
    # are identical across cores (no union blow-up).
    cnt_cb = np.zeros((NCORES, tot_blocks), np.int64)
    for c in range(NCORES):
        cnt_cb[c] = np.bincount(g_blk[c_src == c], minlength=tot_blocks)
    mx_b = cnt_cb.max(axis=0)                # slots per block
    tiles_per_sup = np.zeros(n_supers, np.int64)
    blk_off = np.zeros(tot_blocks, np.int64)  # slot offset within super
    for s in range(n_supers):
        mb = mx_b[s * SUPER:(s + 1) * SUPER]
        blk_off[s * SUPER:(s + 1) * SUPER] = \
            np.concatenate([[0], np.cumsum(mb)[:-1]])
        tiles_per_sup[s] = max(1, -(-int(mb.sum()) // 128))

    # Supers are processed in bucket-third order so each third's
    # ReduceScatter overlaps the remaining aggregation.
    blocks = d["blocks"]
    pb = [blocks - 2 * (blocks // 3), blocks // 3, blocks // 3]  # 34/33/33
    poff = [0, pb[0], pb[0] + pb[1]]

    def _part_of_j(j):
        return 0 if j < poff[1] else (1 if j < poff[2] else 2)

    def _grp(s):
        return min(_part_of_j(b % blocks)
                   for b in range(s * SUPER, (s + 1) * SUPER))

    sup_order = sorted(range(n_supers), key=_grp)
    grp_last = {}  # group -> last position
    for p, s in enumerate(sup_order):
        grp_last[_grp(s)] = p
    pos_of = np.zeros(n_supers, np.int64)
    for p, s in enumerate(sup_order):
        pos_of[s] = p
    tiles_s = np.array([tiles_per_sup[s] for s in sup_order], np.int64)
    tile_off = np.concatenate([[0], np.cumsum(tiles_s)])
    tot_tiles = int(tiles_s.sum())

    idx_planes = []
    rel_planes_flat = []
    for c in range(NCORES):
        m = c_src == c
        e_dst = g_dst[m]
        e_src = src_loc[m]
        sort = np.argsort(e_dst, kind="stable")
        es = e_dst[sort]
        ss = e_src[sort]
        eb = es // 128
        grp = np.searchsorted(eb, eb)
        within = np.arange(eb.size) - grp     # rank within block
        slot = (tile_off[pos_of[eb // SUPER]] * 128 + blk_off[eb] + within)
        assert slot.max() < tot_tiles * 128
        idx_flat = np.zeros(tot_tiles * 128, np.int16)
        rel_flat = np.full(tot_tiles * 128, -1.0, np.float32)
        idx_flat[slot] = ss.astype(np.int16)
        rel_flat[slot] = (es % 128).astype(np.float32)
        idx16 = idx_flat.reshape(tot_tiles * 8, 16).T
        idx_planes.append(np.tile(idx16, (8, 1)).copy())
        rel_planes_flat.append(rel_flat)

    # instances: for each tile, the blocks whose slot range intersects it
    # (shared across cores).  Grouped per block for start/stop flags.
    sup_insts = []  # per POSITION: list of (tile_global, bi)
    n_inst = 0
    for p, s in enumerate(sup_order):
        insts = []
        for bi in range(SUPER):
            b = s * SUPER + bi
            lo, hi = blk_off[b], blk_off[b] + max(1, mx_b[b])
            tlast = int(tile_off[p + 1]) - 1
            t0 = min(int(tile_off[p] + lo // 128), tlast)
            t1 = min(int(tile_off[p] + (hi - 1) // 128), tlast)
            for t in range(t0, t1 + 1):
                insts.append((t, bi))
        sup_insts.append(insts)
        n_inst += len(insts)

    rel_planes = []
    for c in range(NCORES):
        rf = rel_planes_flat[c].reshape(tot_tiles, 128)
        rel_plane = np.full((128, n_inst), -1.0, np.float32)
        j = 0
        for p, s in enumerate(sup_order):
            for (t, bi) in sup_insts[p]:
                b = s * SUPER + bi
                # slots of this tile belonging to block b and this core
                lo = int(tile_off[p] * 128 + blk_off[b])
                hi = lo + int(cnt_cb[c, b])
                p0 = max(0, lo - t * 128)
                p1 = max(0, min(128, hi - t * 128))
                if p1 > p0:
                    rel_plane[p0:p1, j] = rf[t, p0:p1]
                j += 1
        rel_planes.append(np.ascontiguousarray(rel_plane))

    meta = dict(d=d, tiles_s=tiles_s, tile_off=tile_off,
                tot_tiles=tot_tiles, sup_insts=sup_insts, n_inst=n_inst,
                sup_order=sup_order, grp_last=grp_last,
                pb=pb, poff=poff)
    per_core = [dict(idx_plane=idx_planes[c], rel_plane=rel_planes[c])
                for c in range(NCORES)]
    return meta, per_core


def _host_inputs(x, edge_index, W1, b1, W2, b2, cfg):
    d = _derive(cfg)
    bucket, blocks, shard = d["bucket"], d["blocks"], d["shard"]
    n, cin = cfg["N"], cfg["CIN"]
    chid, cout = cfg["CHID"], cfg["COUT"]
    meta, per_core = _preprocess(edge_index, cfg)

    col = edge_index[1].astype(np.int64)
    deg = (np.bincount(col, minlength=n) + 1).astype(np.float32)

    w1 = np.ascontiguousarray(np.asarray(W1, np.float32))
    w2p = np.zeros((chid, 128), np.float32)
    w2p[:, :cout] = np.asarray(W2, np.float32)
    b1row = np.ascontiguousarray(
        np.broadcast_to(np.asarray(b1, np.float32)[None, :], (128, chid)))
    b2row = np.ascontiguousarray(
        np.broadcast_to(np.asarray(b2, np.float32)[None, :], (128, cout)))
    iota = np.ascontiguousarray(
        np.broadcast_to(np.arange(128, dtype=np.float32)[None, :],
                        (128, 128)))
    eye = np.eye(128, dtype=np.float32)

    in_maps = []
    for c in range(NCORES):
        xs = np.zeros((shard, cin), np.float32)
        xs[:bucket] = x[c * bucket:(c + 1) * bucket]
        x_ct = np.ascontiguousarray(xs.T)          # [cin, shard]
        degs = np.ones(shard, np.float32)
        degs[:bucket] = deg[c * bucket:(c + 1) * bucket]
        deg_pm = np.ascontiguousarray(degs.reshape(blocks, 128).T)
        in_maps.append({
            "x_ct": x_ct, "deg_pm": deg_pm,
            "idx_plane": per_core[c]["idx_plane"],
            "rel_plane": per_core[c]["rel_plane"],
            "w1": w1, "w2p": w2p, "b1row": b1row, "b2row": b2row,
            "iota": iota, "eye": eye,
        })
    return meta, in_maps


def _build_program(cfg, meta):
    import concourse.bacc as bacc
    import concourse.mybir as mybir
    from concourse import tile

    d = meta["d"]
    blocks, shard = d["blocks"], d["shard"]
    tot_rows, tot_blocks = d["tot_rows"], d["tot_blocks"]
    n_supers = d["n_supers"]
    tiles_s = meta["tiles_s"]
    tile_off = meta["tile_off"]
    tot_tiles = meta["tot_tiles"]
    sup_insts = meta["sup_insts"]
    n_inst = meta["n_inst"]
    sup_order = meta["sup_order"]
    grp_last = meta["grp_last"]
    pb = meta["pb"]
    poff = meta["poff"]
    cin, chid, cout = cfg["CIN"], cfg["CHID"], cfg["COUT"]

    def _part_of_j(j):
        return 0 if j < poff[1] else (1 if j < poff[2] else 2)

    def evict_segments(s):
        """Super s's 8 blocks -> runs of (part_id, rowblk_in_part, bi0, n)."""
        segs = []
        for bi in range(SUPER):
            b = s * SUPER + bi
            j = b % blocks
            hid = _part_of_j(j)
            rowblk = (b // blocks) * pb[hid] + j - poff[hid]
            if segs and segs[-1][0] == hid and \
                    segs[-1][1] + segs[-1][3] == rowblk:
                segs[-1][3] += 1
            else:
                segs.append([hid, rowblk, bi, 1])
        return segs

    bf16 = mybir.dt.bfloat16
    f32 = mybir.dt.float32
    i16 = mybir.dt.int16
    mult = mybir.AluOpType.mult
    add = mybir.AluOpType.add
    iseq = mybir.AluOpType.is_equal
    act_copy = mybir.ActivationFunctionType.Copy
    act_relu = mybir.ActivationFunctionType.Relu

    nc = bacc.Bacc("TRN2", target_bir_lowering=False, debug=False,
                   num_devices=NCORES)

    x_ct = nc.dram_tensor("x_ct", [cin, shard], f32, kind="ExternalInput")
    deg_pm_t = nc.dram_tensor("deg_pm", [128, blocks], f32,
                              kind="ExternalInput")
    idxp_t = nc.dram_tensor("idx_plane", [128, tot_tiles * 8], i16,
                            kind="ExternalInput")
    relp_t = nc.dram_tensor("rel_plane", [128, n_inst], f32,
                            kind="ExternalInput")
    w1_t = nc.dram_tensor("w1", [cin, chid], f32, kind="ExternalInput")
    w2p_t = nc.dram_tensor("w2p", [chid, 128], f32, kind="ExternalInput")
    b1r_t = nc.dram_tensor("b1row", [128, chid], f32, kind="ExternalInput")
    b2r_t = nc.dram_tensor("b2row", [128, cout], f32, kind="ExternalInput")
    iota_t = nc.dram_tensor("iota", [128, 128], f32, kind="ExternalInput")
    eye_t = nc.dram_tensor("eye", [128, 128], f32, kind="ExternalInput")
    out_t = nc.dram_tensor("out", [shard, cout], f32, kind="ExternalOutput")

    # gather batches: SBATCH supers per dma_gather call
    batches = []
    for s0 in range(0, n_supers, SBATCH):
        ss = list(range(s0, min(s0 + SBATCH, n_supers)))
        t0 = int(tile_off[ss[0]])
        bt = int(sum(tiles_s[s] for s in ss))
        i0 = sum(len(sup_insts[s]) for s in range(ss[0]))
        ni = sum(len(sup_insts[s]) for s in ss)
        batches.append(dict(ss=ss, t0=t0, bt=bt, i0=i0, ni=ni))

    with tile.TileContext(nc) as tc:
        with (
            tc.tile_pool(name="dram", bufs=1, space="DRAM") as dram,
            tc.tile_pool(name="const", bufs=1) as cp,
            tc.tile_pool(name="g2sp", bufs=1) as shp,
            tc.tile_pool(name="xin", bufs=2) as xp,
            tc.tile_pool(name="gout", bufs=3) as gp,
            tc.tile_pool(name="idxs", bufs=3) as ip,
            tc.tile_pool(name="rels", bufs=3) as rp,
            tc.tile_pool(name="stage", bufs=3) as stp,
            tc.tile_pool(name="masks", bufs=8) as mp,
            tc.tile_pool(name="evict", bufs=4) as ep,
            tc.tile_pool(name="work", bufs=4) as wp,
            tc.tile_pool(name="rsin", bufs=2) as rsp,
            tc.tile_pool(name="outp", bufs=4) as op_,
            tc.tile_pool(name="pbig", bufs=4, space="PSUM") as pbig,
            tc.tile_pool(name="pph1", bufs=2, space="PSUM") as pph1,
            tc.tile_pool(name="pptr", bufs=1, space="PSUM") as pptr,
            tc.tile_pool(name="ppg", bufs=1, space="PSUM") as ppg,
        ):
            # ---- DRAM scratch ----
            tab1 = dram.tile([shard, chid], bf16, name="tab1", tag="tab1")
            tab2 = dram.tile([shard, 128], bf16, name="tab2", tag="tab2")
            part1 = [dram.tile([NCORES * pb[h] * 128, chid], bf16,
                               name=f"p1{h}", tag=f"p1{h}")
                     for h in range(3)]
            part2 = [dram.tile([NCORES * pb[h] * 128, cout], bf16,
                               name=f"p2{h}", tag=f"p2{h}")
                     for h in range(3)]
            rs1 = [dram.tile([pb[h] * 128, chid], bf16, name=f"rs1{h}",
                             tag=f"rs1{h}") for h in range(3)]
            rs2 = [dram.tile([pb[h] * 128, cout], bf16, name=f"rs2{h}",
                             tag=f"rs2{h}") for h in range(3)]

            # ---- constants ----
            iota_sb = cp.tile([128, 128], bf16)
            nc.gpsimd.dma_start(iota_sb[:], iota_t[:])  # cast f32->bf16
            eye_sb = cp.tile([128, 128], bf16)
            nc.gpsimd.dma_start(eye_sb[:], eye_t[:])
            w1_sb = cp.tile([cin, chid], bf16)
            nc.gpsimd.dma_start(w1_sb[:], w1_t[:])
            w2_sb = cp.tile([chid, 128], bf16)
            nc.gpsimd.dma_start(w2_sb[:], w2p_t[:])
            b1r_sb = cp.tile([128, chid], f32)
            nc.sync.dma_start(b1r_sb[:], b1r_t[:])
            b2r_sb = cp.tile([128, cout], f32)
            nc.sync.dma_start(b2r_sb[:], b2r_t[:])
            deg_pm = cp.tile([128, blocks], f32)
            nc.sync.dma_start(deg_pm[:], deg_pm_t[:])
            invd_pm = cp.tile([128, blocks], f32)
            nc.scalar.sqrt(invd_pm[:], deg_pm[:])
            dis_pm = cp.tile([128, blocks], f32)
            nc.vector.reciprocal(dis_pm[:], invd_pm[:])

            g2s = shp.tile([128, blocks * 128], bf16)
            g1s = shp.tile([128, blocks * chid], bf16)

            # ---- mini phase 1: own transform -> local g1 table ----
            # g1 stays resident in SBUF (g1s) for the post-RS self-loop add.
            for g in range(blocks // 4):
                if g % 5 == 0:
                    xc = xp.tile([cin, 5 * 512], bf16, tag="xc")
                    c0 = g * 512
                    nc.gpsimd.dma_start(xc[:], x_ct[:, c0:c0 + 5 * 512])
                xoff = (g % 5) * 512
                bank = pph1.tile([128, 512], f32, tag="ph1")
                for k in range(4):
                    nc.tensor.matmul(
                        bank[:, k * 128:(k + 1) * 128],
                        xc[:, xoff + k * 128:xoff + (k + 1) * 128],
                        w1_sb[:], start=True, stop=True)
                gsb = g1s[:, g * 512:(g + 1) * 512]
                for k in range(4):
                    b = g * 4 + k
                    dst = gsb[:, k * 128:(k + 1) * 128]
                    src = bank[:, k * 128:(k + 1) * 128]
                    if g % 2 == 0:
                        nc.vector.tensor_scalar(
                            dst, src, dis_pm[:, b:b + 1], None, mult)
                    else:
                        nc.scalar.activation(
                            dst, src, act_copy, scale=dis_pm[:, b:b + 1])
                nc.sync.dma_start(
                    tab1[g * 512:(g + 1) * 512, :]
                    .rearrange("(b p) c -> p b c", p=128),
                    gsb.rearrange("p (b c) -> p b c", c=chid))

            # ---- push-side aggregation (both layers share planes) ----
            def aggregate(layer):
                tab = tab1 if layer == 1 else tab2
                part = part1 if layer == 1 else part2
                rsout = rs1 if layer == 1 else rs2
                ocols = chid if layer == 1 else cout
                nbank = (SUPER * ocols * 4 - 1) // 2048 + 1  # banks/super
                bw = SUPER * ocols // nbank  # psum cols per bank
                for bat in batches:
                    idxs = ip.tile([128, bat["bt"] * 8], i16, tag="idx")
                    nc.sync.dma_start(
                        idxs[:],
                        idxp_t[:, bat["t0"] * 8:(bat["t0"] + bat["bt"]) * 8])
                    rels = rp.tile([128, bat["ni"]], f32, tag="rel")
                    nc.sync.dma_start(
                        rels[:],
                        relp_t[:, bat["i0"]:bat["i0"] + bat["ni"]])
                    st = stp.tile([128, bat["bt"], 128], bf16, tag="stage")
                    nc.gpsimd.dma_gather(
                        st[:], tab[:], idxs[:], bat["bt"] * 128,
                        bat["bt"] * 128, 128, single_packet=False)
                    jo = 0
                    for s in bat["ss"]:
                        psums = [pbig.tile([128, bw], f32, name="acc",
                                           tag="acc") for _ in range(nbank)]

                        def pacc(bi):
                            o = bi * ocols
                            return psums[o // bw][:, o % bw:o % bw + ocols]

                        insts = sup_insts[s]
                        first = {}
                        last = {}
                        for j, (t, bi) in enumerate(insts):
                            if bi not in first:
                                first[bi] = j
                            last[bi] = j
                        for j, (t, bi) in enumerate(insts):
                            col = bat["i0"] + jo + j
                            mk = mp.tile([128, 128], bf16, tag="mask")
                            nc.vector.tensor_scalar(
                                mk[:], iota_sb[:],
                                rels[:, col - bat["i0"]:col - bat["i0"] + 1],
                                None, iseq)
                            tl = t - bat["t0"]
                            nc.tensor.matmul(
                                pacc(bi), mk[:],
                                st[:, tl:tl + 1, :ocols].squeeze(),
                                start=(first[bi] == j),
                                stop=(last[bi] == j))
                        jo += len(insts)
                        # evict whole super -> partial rows (no scale/bias)
                        ev = ep.tile([128, SUPER * ocols], bf16, tag="ev")
                        for k in range(nbank):
                            dst = ev[:, k * bw:(k + 1) * bw]
                            if s % 2 == 0:
                                nc.vector.tensor_copy(dst, psums[k][:])
                            else:
                                nc.scalar.activation(dst, psums[k][:],
                                                     act_copy)
                        for hid, rowblk, bi0, nb in evict_segments(
                                sup_order[s]):
                            nc.sync.dma_start(
                                part[hid][rowblk * 128:
                                          (rowblk + nb) * 128, :]
                                .rearrange("(b p) c -> p b c", p=128),
                                ev[:, bi0 * ocols:(bi0 + nb) * ocols]
                                .rearrange("p (b c) -> p b c", c=ocols))
                    for g_ in (0, 1):
                        if grp_last[g_] in bat["ss"]:
                            # this part's partials are complete: start its
                            # RS now, overlapping remaining aggregation
                            nc.gpsimd.collective_compute(
                                "ReduceScatter", add,
                                replica_groups=[list(range(NCORES))],
                                ins=[part[g_].opt()],
                                outs=[rsout[g_].opt()])
                nc.gpsimd.collective_compute(
                    "ReduceScatter", add,
                    replica_groups=[list(range(NCORES))],
                    ins=[part[2].opt()], outs=[rsout[2].opt()])
                return rsout

            # ---- layer 1 ----
            rs1o = aggregate(1)
            # post: h1 = relu(dis*rs + b1); g2 = dis*(h1@W2) -> tab2
            for h in range(3):
                nb2 = pb[h] // 2
                rsb = rsp.tile([128, pb[h] * chid], bf16, tag="rsb")
                for hh in range(2):  # chunked: >4k descs/DMA crash
                    h0 = hh * nb2
                    nbc = nb2 if hh == 0 else pb[h] - nb2
                    nc.sync.dma_start(
                        rsb[:, h0 * chid:(h0 + nbc) * chid]
                        .rearrange("p (b c) -> p b c", c=chid),
                        rs1o[h][h0 * 128:(h0 + nbc) * 128, :]
                        .rearrange("(b p) c -> p b c", p=128))
                for j in range(pb[h]):
                    b = poff[h] + j
                    rblk = rsb[:, j * chid:(j + 1) * chid]
                    t1 = wp.tile([128, chid], f32, tag="t1")
                    nc.vector.tensor_tensor(
                        t1[:], rblk, g1s[:, b * chid:(b + 1) * chid], add)
                    t2 = wp.tile([128, chid], f32, tag="t2")
                    nc.vector.scalar_tensor_tensor(
                        t2[:], t1[:], dis_pm[:, b:b + 1], b1r_sb[:],
                        mult, add)
                    h1r = wp.tile([128, chid], bf16, tag="h1r")
                    nc.scalar.activation(h1r[:], t2[:], act_relu)
                    ptr = pptr.tile([128, chid], bf16, tag="ptr")
                    nc.tensor.transpose(ptr[:], h1r[:], eye_sb[:])
                    ht = wp.tile([128, chid], bf16, tag="ht")
                    nc.scalar.activation(ht[:], ptr[:], act_copy)
                    pg = ppg.tile([128, 128], f32, tag="pg")
                    nc.tensor.matmul(pg[:], ht[:], w2_sb[:],
                                     start=True, stop=True)
                    nc.vector.tensor_scalar(
                        g2s[:, b * 128:(b + 1) * 128], pg[:],
                        dis_pm[:, b:b + 1], None, mult)
                    if b % 4 == 3 or j == pb[h] - 1:
                        b0 = (b // 4) * 4
                        nc.sync.dma_start(
                            tab2[b0 * 128:(b + 1) * 128, :]
                            .rearrange("(b p) c -> p b c", p=128),
                            g2s[:, b0 * 128:(b + 1) * 128]
                            .rearrange("p (b c) -> p b c", c=128))

            # ---- layer 2 ----
            rs2o = aggregate(2)
            for h in range(3):
                rsb2 = rsp.tile([128, pb[h] * cout], bf16, tag="rsb2")
                nc.sync.dma_start(
                    rsb2[:].rearrange("p (b c) -> p b c", c=cout),
                    rs2o[h][:].rearrange("(b p) c -> p b c", p=128))
                for j4 in range(-(-pb[h] // 4)):
                    n4 = min(4, pb[h] - j4 * 4)
                    ob = op_.tile([128, 4 * cout], f32, tag="ob")
                    for k in range(n4):
                        j = j4 * 4 + k
                        b = poff[h] + j
                        rblk = rsb2[:, j * cout:(j + 1) * cout]
                        t1 = wp.tile([128, cout], f32, tag="o1")
                        nc.vector.tensor_tensor(
                            t1[:], rblk,
                            g2s[:, b * 128:b * 128 + cout], add)
                        nc.vector.scalar_tensor_tensor(
                            ob[:, k * cout:(k + 1) * cout], t1[:],
                            dis_pm[:, b:b + 1], b2r_sb[:], mult, add)
                    r0 = (poff[h] + j4 * 4) * 128
                    nc.sync.dma_start(
                        out_t[r0:r0 + n4 * 128, :]
                        .rearrange("(b p) c -> p b c", p=128),
                        ob[:, :n4 * cout]
                        .rearrange("p (b c) -> p b c", c=cout))

    nc.compile()
    return nc


def run_config(inputs, cfg, run=None):
    from concourse.bass_utils import run_bass_kernel_spmd

    x = np.asarray(inputs["x"], np.float32)
    edge_index = np.asarray(inputs["edge_index"])
    meta, in_maps = _host_inputs(
        x, edge_index, inputs["W1"], inputs["b1"], inputs["W2"],
        inputs["b2"], cfg)
    nc = _build_program(cfg, meta)
    if run is None:
        def run(nc, in_maps):
            return run_bass_kernel_spmd(
                nc, in_maps, list(range(NCORES))).results
    results = run(nc, in_maps)
    bucket = _derive(cfg)["bucket"]
    out = np.concatenate(
        [results[c]["out"][:bucket] for c in range(NCORES)], axis=0)
    return np.ascontiguousarray(out.astype(np.float32))


def kernel(**inputs):
    return run_config(inputs, CFG_FULL)
